# revision 50
# baseline (speedup 1.0000x reference)
"""Trainium2 Bass kernel for Chebyshev (L-inf) "convolution".

Math (see reference):
  out[b,co,h,w] = max_n |weights[co,n] - x_pad[b, c(co,n), h+di(co,n), w+dj(co,n)]| + bias[co]
  where conn_idx[co,n] = c*9 + di*3 + dj and x_pad is replicate-padded by 1.

Strategy (8 NeuronCores, batch-sharded: 4 images per core):
  Host prep: replicate-pad x to [B, 64, 66, 66], cast to bf16, and ship the
  flattened per-core shard as the gather source ("xg", 2.23 MB/core).  All
  per-(co,n) gather offsets (element offsets into xg) are host-computed from
  conn_idx.  No on-device pad/cast/store stage: the 16 indirect gathers
  depend only on an external input + the offset table (loaded on the gpsimd
  queue itself) and fire right after program start, so the DMA bus - the
  binding resource at ~21.5 MB/core - is busy almost immediately.

  Device, per (image, tap): one indirect DMA (GPSIMD ucode, 128
  descriptors): partition co reads the contiguous 4222-element bf16 span of
  xg at off = b*278784 + c*4356 + di*66 + dj; the shifted 64x64 window sits
  at row-stride 66 inside the span.  Compute per image, balanced so both
  elementwise engines run ~11 us/image, just under the ~12 us/image DMA-bus
  cadence:
    - taps 0-2 on ScalarE: t = Abs(g + (-w)) activation (per-partition bias)
    - tap 3 on DVE: tensor_scalar subtract (4x bf16 mode) + sign-clear via
      bitwise_and 0x7FFF on the u16 view (exact bf16 abs, 4x mode)
    - DVE max tree (2x bf16) + per-partition bias add, store bf16
      (host upcasts to fp32; bf16 rounding ~0.4% << the 2e-2 tolerance).

  The per-image max tree is a chain (max(max(max(t0,t1),t2),t3)) so only
  one DVE op depends on the image's last-arriving gather; the final image
  additionally streams its DVE tap + final max/bias/store at half-plane
  granularity, and its last gather is itself split into two half-spans so
  the first half-chain overlaps the second half's transfer.  Image 0 runs
  1 ScalarE tap + 3 DVE taps: DVE is otherwise idle during the ~16 us head
  while ScalarE is the saturated stream.

  Per-core DRAM traffic: 17.3 MB gather + 4.2 MB out (vs 32.1 MB baseline).
  ~75-77 us in quiet windows (vs 122 us baseline); device-wide HBM
  contention can add ~10 us run-to-run.  The residual floor is the shared
  ~360-450 GB/s per-core DMA bus moving the 8x gather expansion (128
  out-channels each pulling private copies of 4 input planes), which no
  on-chip engine can replicate cheaper: indirect DMA is DRAM-source-only,
  SBUF<->SBUF DMA shares the same bus, PE permutation cannot apply the
  per-channel window shift, and GPSIMD gathers are ~5x slower than DVE.
  Both elementwise engines run ~11 us/image against the ~12 us/image bus
  cadence, so compute stays hidden.  Known-bad variants (measured): Pool
  tensor ops (invalid on TRN2), abs_max ALU op (no walrus lowering),
  multi-offset indirect DMA (device error), chunked middle images (DVE
  backlog), sub-tile cross-engine half-act handoffs (missed hazard edges
  -> races), splitting the first gather into half-spans (the extra ucode
  issue delays all later gathers), last image at 2S/2V (extra mid-stream
  DVE work outweighs the shorter tail), late-dependency ops (e.g. bias
  Identity act) queued mid-stream on the in-order ScalarE (+4 us cascade
  stall), quarter-granularity tails (tail is DVE-engine-serial; finer ops
  add overhead without shortening it), stride-4-swizzled descriptor order
  for SBUF-port balancing (2D partition APs on indirect DMA -> device
  error + wedged core; recover with NEURON_RT_RESET_CORES=1).  Bacc's scheduler may reorder
  per-engine streams - emission order is a hint, not a schedule.

  Run-to-run HW variance is bimodal (~75 vs ~85 us) from device-wide HBM
  contention; compare configs by fast-cluster minimum over >=5 reps.
  Final measured distribution (5 batches x 5 reps): fast cluster
  74.84-77.4 us, best 74.84 us, slow-window outliers 83-93 us.
  (Do NOT confuse this last-gather split with the known-bad FIRST-gather
  split above - only early splits delay later ucode issues.)
"""

import numpy as np

B, CIN, H, W = 32, 64, 64, 64
COUT, NCONN = 128, 4
KH, KW = 3, 3
NCORES = 8
BL = B // NCORES            # 4 images per core
PH, PW = H + 2, W + 2       # 66 x 66 replicate-padded planes
PLANE = PH * PW             # 4356
IMG = CIN * PLANE           # 278784 elements per padded image
S = H * W                   # 4096
SPAN = (H - 1) * PW + W     # 4222: span holding one shifted 64x64 window
GPAD = 64 * PW              # 4224 (= 64*66) per-tap stride in the G tile

_CACHE = {}


def _build_program():
    import concourse.bass as bass
    import concourse.bacc as bacc
    import concourse.mybir as mybir
    from concourse.tile import TileContext

    f32 = mybir.dt.float32
    bf16 = mybir.dt.bfloat16
    u16 = mybir.dt.uint16
    i32 = mybir.dt.int32
    Alu = mybir.AluOpType
    Act = mybir.ActivationFunctionType

    nc = bacc.Bacc("TRN2", target_bir_lowering=False, debug=False)

    # flattened padded bf16 images for this core's 4-image shard
    xg = nc.dram_tensor("xg", (BL * IMG, 1), bf16, kind="ExternalInput")
    w_ext = nc.dram_tensor("w", (COUT, NCONN), f32, kind="ExternalInput").ap()
    wneg_ext = nc.dram_tensor("wneg", (COUT, NCONN), f32, kind="ExternalInput").ap()
    bias_ext = nc.dram_tensor("bias", (COUT, 1), f32, kind="ExternalInput").ap()
    # per (b, n): one 8-int32 slot per partition at cols [(b*NCONN+n)*8, +8);
    # the indirect-DMA ucode reads col 0 of each 32 B slot.
    gidx_ext = nc.dram_tensor(
        "gidx", (COUT, (BL * NCONN + 1) * 8), i32, kind="ExternalInput"
    ).ap()
    out_ext = [
        nc.dram_tensor(f"out{b}", (COUT, S), bf16, kind="ExternalOutput").ap()
        for b in range(BL)
    ]

    with TileContext(nc, pool_alloc_mode="queue") as tc:
        with (
            tc.tile_pool(name="const", bufs=1) as cpool,
            tc.tile_pool(name="g", bufs=12) as gpool,
            tc.tile_pool(name="t", bufs=6) as tpool,
            tc.tile_pool(name="m", bufs=3) as mpool,
            tc.tile_pool(name="m2", bufs=2) as m2pool,
            tc.tile_pool(name="o", bufs=2) as opool,
        ):
            # gidx gates the gathers: load it on the gpsimd queue itself so
            # the first indirect DMA needs no cross-engine wait.
            gidx_sb = cpool.tile([COUT, (BL * NCONN + 1) * 8], i32)
            nc.gpsimd.dma_start(out=gidx_sb[:], in_=gidx_ext)
            w_sb = cpool.tile([COUT, NCONN], f32)
            nc.sync.dma_start(out=w_sb[:], in_=w_ext)
            wneg_sb = cpool.tile([COUT, NCONN], f32)
            nc.sync.dma_start(out=wneg_sb[:], in_=wneg_ext)
            bias_sb = cpool.tile([COUT, 1], f32)
            nc.sync.dma_start(out=bias_sb[:], in_=bias_ext)
            absmask_sb = cpool.tile([COUT, 1], u16)
            nc.vector.memset(absmask_sb[:], 0x7FFF)

            # offset view [COUT, BL*NCONN, 1] with 8-int32 slot stride
            gidx_v = gidx_sb[:].rearrange(
                "p (k e) -> p k e", k=BL * NCONN + 1, e=8
            )

            # --- all gathers up front.  The very LAST gather (img 3,
            #     tap 3) is split into two half-row-block spans (slot 16
            #     holds off+HSPLIT) so the tail's first half-chain overlaps
            #     the second half's transfer; the extra ucode issue is at
            #     the end of the issue stream where it delays nothing. ---
            HSPLIT = (H // 2) * PW            # 2112
            HSPAN = (H // 2 - 1) * PW + W     # 2110
            gts = []
            for b in range(BL):
                row = []
                for n in range(NCONN):
                    k = b * NCONN + n
                    gt = gpool.tile([COUT, GPAD], bf16, tag="g")
                    if b == BL - 1 and n == NCONN - 1:
                        nc.gpsimd.indirect_dma_start(
                            out=gt[:, 0:HSPAN],
                            out_offset=None,
                            in_=xg.ap(),
                            in_offset=bass.IndirectOffsetOnAxis(
                                ap=gidx_v[:, k : k + 1, 0:1], axis=0
                            ),
                        )
                        nc.gpsimd.indirect_dma_start(
                            out=gt[:, HSPLIT : HSPLIT + HSPAN],
                            out_offset=None,
                            in_=xg.ap(),
                            in_offset=bass.IndirectOffsetOnAxis(
                                ap=gidx_v[
                                    :, BL * NCONN : BL * NCONN + 1, 0:1
                                ],
                                axis=0,
                            ),
                        )
                    else:
                        nc.gpsimd.indirect_dma_start(
                            out=gt[:, 0:SPAN],
                            out_offset=None,
                            in_=xg.ap(),
                            in_offset=bass.IndirectOffsetOnAxis(
                                ap=gidx_v[:, k : k + 1, 0:1], axis=0
                            ),
                        )
                    row.append(gt)
                gts.append(row)

            # --- compute: 3 Abs taps on ScalarE, 1 on DVE; max tree +
            #     bias on DVE; store bf16 out ---
            def process(b, n_scalar, chunks):
                HC = H // chunks
                L = HC * W
                for ch in range(chunks):
                    rs = slice(ch * HC, (ch + 1) * HC)
                    fs = slice(ch * L, (ch + 1) * L)
                    ts = []
                    for n in range(NCONN):
                        gv = gts[b][n][:].rearrange(
                            "p (h w) -> p h w", h=H, w=PW
                        )[:, rs, 0:W]
                        tt = tpool.tile([COUT, S], bf16, tag="t")
                        tv = tt[:, 0:L].rearrange(
                            "p (h w) -> p h w", h=HC, w=W
                        )
                        if n < n_scalar:
                            nc.scalar.activation(
                                out=tv,
                                in_=gv,
                                func=Act.Abs,
                                bias=wneg_sb[:, n : n + 1],
                                scale=1.0,
                            )
                        else:
                            nc.vector.tensor_scalar(
                                out=tv,
                                in0=gv,
                                scalar1=w_sb[:, n : n + 1],
                                scalar2=None,
                                op0=Alu.subtract,
                            )
                            nc.vector.tensor_scalar(
                                out=tt[:, 0:L].bitcast(u16),
                                in0=tt[:, 0:L].bitcast(u16),
                                scalar1=absmask_sb[:, 0:1],
                                scalar2=None,
                                op0=Alu.bitwise_and,
                            )
                        ts.append(tt)
                    m01 = mpool.tile([COUT, S], bf16, tag="m")
                    nc.vector.tensor_tensor(
                        out=m01[:, 0:L], in0=ts[0][:, 0:L], in1=ts[1][:, 0:L],
                        op=Alu.max,
                    )
                    m012 = mpool.tile([COUT, S], bf16, tag="m")
                    nc.vector.tensor_tensor(
                        out=m012[:, 0:L], in0=m01[:, 0:L], in1=ts[2][:, 0:L],
                        op=Alu.max,
                    )
                    mf = m2pool.tile([COUT, S], bf16, tag="m2")
                    nc.vector.tensor_tensor(
                        out=mf[:, 0:L], in0=m012[:, 0:L], in1=ts[3][:, 0:L],
                        op=Alu.max,
                    )
                    ob = opool.tile([COUT, S], bf16, tag="o")
                    nc.vector.tensor_scalar(
                        out=ob[:, 0:L],
                        in0=mf[:, 0:L],
                        scalar1=bias_sb[:, 0:1],
                        scalar2=None,
                        op0=Alu.add,
                    )
                    nc.sync.dma_start(out=out_ext[b][:, fs], in_=ob[:, 0:L])

            def process_last(b):
                # Last image: only one DVE op may depend on its final gather.
                # Taps 0-2 (ScalarE) chain into m012 early; the DVE tap 3 and
                # the final max/bias/store stream in half-planes.
                ts = []
                for n in range(3):
                    gv = gts[b][n][:].rearrange(
                        "p (h w) -> p h w", h=H, w=PW
                    )[:, :, 0:W]
                    tt = tpool.tile([COUT, S], bf16, tag="t")
                    tv = tt[:].rearrange("p (h w) -> p h w", h=H, w=W)
                    nc.scalar.activation(
                        out=tv, in_=gv, func=Act.Abs,
                        bias=wneg_sb[:, n : n + 1], scale=1.0,
                    )
                    ts.append(tt)
                m01 = mpool.tile([COUT, S], bf16, tag="m")
                nc.vector.tensor_tensor(
                    out=m01[:], in0=ts[0][:], in1=ts[1][:], op=Alu.max
                )
                # tap 3 (DVE) in halves, emitted before the tail chain so the
                # DVE queue reaches the tail ops as soon as abs2 retires
                t3 = tpool.tile([COUT, S], bf16, tag="t")
                HHALF = S // 2
                for hh in range(2):
                    hs = slice(hh * HHALF, (hh + 1) * HHALF)
                    gv = gts[b][3][:].rearrange(
                        "p (h w) -> p h w", h=H, w=PW
                    )[:, hh * (H // 2) : (hh + 1) * (H // 2), 0:W]
                    tv = t3[:, hs].rearrange(
                        "p (h w) -> p h w", h=H // 2, w=W
                    )
                    nc.vector.tensor_scalar(
                        out=tv, in0=gv, scalar1=w_sb[:, 3:4],
                        scalar2=None, op0=Alu.subtract,
                    )
                    nc.vector.tensor_scalar(
                        out=t3[:, hs].bitcast(u16),
                        in0=t3[:, hs].bitcast(u16),
                        scalar1=absmask_sb[:, 0:1],
                        scalar2=None, op0=Alu.bitwise_and,
                    )
                # tail chain fully at half granularity; every read below is
                # of a fully-written tile, so the hazards are whole-tile
                m012 = mpool.tile([COUT, S], bf16, tag="m")
                mf = m2pool.tile([COUT, S], bf16, tag="m2")
                ob = opool.tile([COUT, S], bf16, tag="o")
                for hh in range(2):
                    hs = slice(hh * HHALF, (hh + 1) * HHALF)
                    nc.vector.tensor_tensor(
                        out=m012[:, hs], in0=m01[:, hs], in1=ts[2][:, hs],
                        op=Alu.max,
                    )
                    nc.vector.tensor_tensor(
                        out=mf[:, hs], in0=m012[:, hs], in1=t3[:, hs],
                        op=Alu.max,
                    )
                    nc.vector.tensor_scalar(
                        out=ob[:, hs], in0=mf[:, hs],
                        scalar1=bias_sb[:, 0:1], scalar2=None, op0=Alu.add,
                    )
                    nc.sync.dma_start(out=out_ext[b][:, hs], in_=ob[:, hs])

            for b in range(BL):
                if b == BL - 1:
                    process_last(b)
                else:
                    # image 0: DVE is otherwise idle during the head while
                    # ScalarE is the saturated stream - give DVE 3 of the
                    # first image's taps
                    process(b, n_scalar=1 if b == 0 else 3, chunks=1)
    nc.compile()
    return nc


def _host_inputs(x, weights, bias, conn_idx):
    """Per-core input maps.  Host prep: replicate-pad + bf16-cast x, shard by
    batch, and derive gather element-offsets from the tiny index tensor."""
    import ml_dtypes

    bf16 = ml_dtypes.bfloat16

    x = np.asarray(x, dtype=np.float32)
    xp = np.pad(x, ((0, 0), (0, 0), (1, 1), (1, 1)), mode="edge")  # [B,64,66,66]
    xpb = xp.astype(bf16).reshape(B, IMG)

    ci = np.asarray(conn_idx).astype(np.int64)          # [COUT, NCONN]
    c = ci // (KH * KW)
    rem = ci % (KH * KW)
    di = rem // KW
    dj = rem % KW
    offs = (c * PLANE + di * PW + dj).astype(np.int32)  # [COUT, NCONN]
    gidx = np.zeros((COUT, (BL * NCONN + 1) * 8), dtype=np.int32)
    for bb in range(BL):
        for n in range(NCONN):
            k = bb * NCONN + n
            gidx[:, k * 8] = bb * IMG + offs[:, n]
    # slot 16: rows 32+ of the last gather (img BL-1, tap 3)
    gidx[:, BL * NCONN * 8] = (BL - 1) * IMG + offs[:, 3] + (H // 2) * PW

    w2 = np.ascontiguousarray(np.asarray(weights), dtype=np.float32)
    bias2 = np.asarray(bias).reshape(COUT, 1).astype(np.float32)
    in_maps = []
    for kcore in range(NCORES):
        in_maps.append(
            {
                "xg": np.ascontiguousarray(
                    xpb[kcore * BL : (kcore + 1) * BL].reshape(BL * IMG, 1)
                ),
                "w": w2,
                "wneg": -w2,
                "bias": bias2,
                "gidx": gidx,
            }
        )
    return in_maps


def kernel(x, weights, bias, conn_idx):
    from concourse.bass_utils import run_bass_kernel_spmd

    if "nc" not in _CACHE:
        _CACHE["nc"] = _build_program()
    nc = _CACHE["nc"]
    in_maps = _host_inputs(x, weights, bias, conn_idx)
    res = run_bass_kernel_spmd(nc, in_maps, list(range(NCORES)))
    outs = [
        np.stack(
            [
                np.asarray(res.results[k][f"out{b}"])
                .astype(np.float32)
                .reshape(COUT, H, W)
                for b in range(BL)
            ]
        )
        for k in range(NCORES)
    ]
    return np.concatenate(outs, axis=0).astype(np.float32)


if __name__ == "__main__":
    nc = _build_program()
    print("program built OK")
